# revision 34
# baseline (speedup 1.0000x reference)
"""Trainium2 Bass kernel for nn_Attention (llama-style attention layer).

Full inputs in, full output out. 8-way tensor-parallel over heads (4 heads
per core, both batches on every core). All matmul operands in bf16 (fp32
PSUM accumulation), which halves HBM traffic and weight-load (LDWEIGHTS)
time vs f32r — the f32r baseline was LDWEIGHTS-bound at ~263ns per 512-row
matmul; bf16 runs at the ~213ns roofline.

  - merged q/k projections per head-pair with RoPE evaluated elementwise on
    DVE straight out of PSUM (features pre-reordered [evens|odds] host-side,
    so no permutation matmul / cross-partition shuffle is needed)
  - v projected directly in [token, feature] layout (x tiles as the matmul
    stationary) so no PE transposes are needed for the attention AV matmul
  - per-head attention in [feat, tok] layout, softmax denominator via
    all-ones matmul, normalization on eviction
  - per-head AllToAll (8 cores, bf16) redistributes attention output from
    head-sharding to token-sharding, overlapped with later heads
  - output projection streams the full wo in four per-head-index passes,
    each gated only on its own AllToAll, accumulating into SBUF partials;
    the passes are emitted last so they fill tensor-engine gaps
"""
import os
import sys

sys.path.insert(0, "/opt/trn_rl_repo")

import numpy as np

import concourse.bass as bass
import concourse.mybir as mybir
import concourse.tile as tile
from concourse import bacc
from concourse.bass import ds, ts
from concourse.bass_utils import run_bass_kernel_spmd

DIM = 4096
N_HEADS = 32
HEAD_DIM = 128
B, S = 2, 2048
TOK = B * S                   # 4096 global tokens
N_CORES = 8
HPC = N_HEADS // N_CORES      # heads per core = 4
FPC = HPC * HEAD_DIM          # features per core = 512
P = 128
KO = DIM // P                 # 32 k-tiles over the model dim
STRIPE = 1024
NSTRIPE = TOK // STRIPE       # 4 projection stripes of 1024 tokens
SCALE = 1.0 / float(np.sqrt(HEAD_DIM))

f32 = mybir.dt.float32
bf16 = mybir.dt.bfloat16
EXP = mybir.ActivationFunctionType.Exp
COPY = mybir.ActivationFunctionType.Copy
MULT = mybir.AluOpType.mult
ADD = mybir.AluOpType.add
SUB = mybir.AluOpType.subtract

_CACHE = {}


def _build():
    nc = bacc.Bacc(
        "TRN2", target_bir_lowering=False, debug=False, num_devices=N_CORES
    )

    xT = nc.dram_tensor("xT", [DIM, TOK], bf16, kind="ExternalInput")
    # q/k weights: per-head-pair stationary tiles, features in [e|o] order
    wqH = nc.dram_tensor("wqH", [HPC, P, KO, P], bf16, kind="ExternalInput")
    wkH = nc.dram_tensor("wkH", [HPC, P, KO, P], bf16, kind="ExternalInput")
    # v weights transposed: [in-part, ktile, out-feats] (moving operand)
    wvT = nc.dram_tensor("wvT", [P, KO, FPC], bf16, kind="ExternalInput")
    woH = nc.dram_tensor("woH", [DIM // P, P, HPC, N_CORES, P], bf16,
                         kind="ExternalInput")
    cb_d = nc.dram_tensor("cb", [P, S], f32, kind="ExternalInput")
    ss_d = nc.dram_tensor("ss", [P, S], f32, kind="ExternalInput")
    ones_d = nc.dram_tensor("ones", [P, P], bf16, kind="ExternalInput")
    out_e = nc.dram_tensor("out", [DIM, TOK // N_CORES], f32, kind="ExternalOutput")

    xT3 = xT.ap().rearrange("(ko p) t -> p ko t", p=P)       # [128, 32, 4096]
    oe3 = out_e.ap().rearrange("(no p) t -> p no t", p=P)    # [128, 32, 512]

    with tile.TileContext(nc) as tc:
        with tc.tile_pool(name="dram", bufs=1, space="DRAM") as drp, \
             tc.tile_pool(name="const", bufs=1) as constp, \
             tc.tile_pool(name="preload", bufs=1) as prep:
            # q/k post-rope, [head][feat e|o][tok-within-batch], bf16;
            # split per batch so batch-0 attention loads don't dep-wait on
            # the batch-1 projection stripes
            q_d = [drp.tile([HPC, P, S], bf16, tag=f"q_d{b}", name=f"q_d{b}")
                   for b in range(B)]
            k_d = [drp.tile([HPC, P, S], bf16, tag=f"k_d{b}", name=f"k_d{b}")
                   for b in range(B)]
            # v stays resident in SBUF for the whole kernel: no DRAM
            # round-trip, and nothing v-related can be stranded in the DMA
            # rings when a collective freezes them
            v_sb = prep.tile([P, B, S // P, FPC], bf16, tag="v_sb",
                             name="v_sb")
            cc_in = [
                drp.tile([N_CORES * P, 512], bf16, tag=f"cci{j}", name=f"cci{j}")
                for j in range(HPC)
            ]
            cc_out = [
                drp.tile([N_CORES * P, 512], bf16, tag=f"cco{j}", name=f"cco{j}")
                for j in range(HPC)
            ]
            cci3 = [c[:].rearrange("(r p) t -> p r t", p=P) for c in cc_in]
            cco3 = [c[:].rearrange("(g p) t -> p g t", p=P) for c in cc_out]
            # scratch target for the tiny gpsimd probe-DMAs that fence each
            # AllToAll behind the next attention unit's loads
            probe_d = drp.tile([1, 64], bf16, tag="probe", name="probe_d")

            ones_sb = constp.tile([P, P], bf16, tag="ones", name="ones_sb")
            nc.sync.dma_start(ones_sb[:], ones_d.ap())

            # ---------- Phase 1: q/k projections (+RoPE) and v^T projection
            with tc.tile_pool(name="p1_rope", bufs=1) as ropep, \
                 tc.tile_pool(name="p1_wv", bufs=1) as wvp, \
                 tc.tile_pool(name="p1_x", bufs=34) as xp, \
                 tc.tile_pool(name="p1_w", bufs=3) as wtp, \
                 tc.tile_pool(name="p1_tmp", bufs=4) as tmpp, \
                 tc.tile_pool(name="p1_qko", bufs=4) as qkop, \
                 tc.tile_pool(name="p1_ps", bufs=3, space="PSUM") as qkps, \
                 tc.tile_pool(name="p1_psv", bufs=2, space="PSUM") as vps:
                cb_sb = ropep.tile([P, S], f32, tag="cb", name="cb_sb")
                ss_sb = ropep.tile([P, S], f32, tag="ss", name="ss_sb")
                wvT_sb = wvp.tile([P, KO, FPC], bf16, tag="wvT", name="wvT_sb")

                for n in range(NSTRIPE):  # 4 stripes of 1024 tokens
                    rtok = (STRIPE * n) % S   # rope tables repeat per batch
                    nb = n // (S // STRIPE)   # batch of this stripe
                    nto = (STRIPE * n) % S    # token offset within batch
                    pairlist = [(wH, dst, a)
                                for wH, dst in ((wqH, q_d), (wkH, k_d))
                                for a in range(HPC // 2)]
                    # interleave weight DMAs with the x-tile stream so each
                    # pair's weights land before its matmuls are reached
                    # (instead of FIFO-queuing behind 8MB of x)
                    wts = {}

                    def emit_wt(pi):
                        wH, _, a = pairlist[pi]
                        wt_e = wtp.tile([P, KO, P], bf16, tag="wt",
                                        name="wt_e")
                        wt_o = wtp.tile([P, KO, P], bf16, tag="wt",
                                        name="wt_o")
                        nc.sync.dma_start(wt_e[:], wH.ap()[2 * a])
                        nc.sync.dma_start(wt_o[:], wH.ap()[2 * a + 1])
                        wts[pi] = (wt_e, wt_o)

                    emit_wt(0)
                    xs = [
                        xp.tile([P, STRIPE], bf16, tag="xsl", name="xs")
                        for _ in range(KO)
                    ]
                    for kl in range(KO):
                        eng = nc.sync if kl % 2 == 0 else nc.scalar
                        eng.dma_start(xs[kl][:], xT3[:, kl, ts(n, STRIPE)])
                        if n == 0 and kl in (7, 15, 23):
                            emit_wt(kl // 8 + 1)
                    if n == 0:
                        # emitted after the first stripe's x/weight DMAs so
                        # they don't delay the first matmul
                        nc.sync.dma_start(cb_sb[:], cb_d.ap())
                        nc.sync.dma_start(ss_sb[:], ss_d.ap())
                        nc.sync.dma_start(wvT_sb[:], wvT.ap())
                    for pi, (wH, dst, a) in enumerate(pairlist):
                            pe = qkps.tile([P, STRIPE], f32, tag="qk",
                                           name="pe")
                            po = qkps.tile([P, STRIPE], f32, tag="qk",
                                           name="po")
                            if pi not in wts:
                                emit_wt(pi)
                            wt_e, wt_o = wts.pop(pi)
                            for kl in range(KO):
                                for c in range(2):
                                    nc.tensor.matmul(
                                        pe[:, ts(c, 512)], wt_e[:, kl],
                                        xs[kl][:, ts(c, 512)],
                                        start=(kl == 0), stop=(kl == KO - 1),
                                    )
                                for c in range(2):
                                    nc.tensor.matmul(
                                        po[:, ts(c, 512)], wt_o[:, kl],
                                        xs[kl][:, ts(c, 512)],
                                        start=(kl == 0), stop=(kl == KO - 1),
                                    )
                            # RoPE: rows 0:64 head 2a, 64:128 head 2a+1
                            # (pe = even-index feats, po = odd-index feats)
                            cbs = cb_sb[:, ds(rtok, STRIPE)]
                            sss = ss_sb[:, ds(rtok, STRIPE)]
                            t1 = tmpp.tile([P, STRIPE], f32, tag="t", name="t1")
                            t2 = tmpp.tile([P, STRIPE], f32, tag="t", name="t2")
                            t3 = tmpp.tile([P, STRIPE], f32, tag="t", name="t3")
                            t4 = tmpp.tile([P, STRIPE], f32, tag="t", name="t4")
                            oe = qkop.tile([P, STRIPE], bf16, tag="o", name="oe")
                            oo = qkop.tile([P, STRIPE], bf16, tag="o", name="oo")
                            nc.vector.tensor_tensor(t1[:], pe[:], cbs, MULT)
                            nc.vector.tensor_tensor(t2[:], po[:], sss, MULT)
                            nc.vector.tensor_tensor(oe[:], t1[:], t2[:], SUB)
                            nc.vector.tensor_tensor(t3[:], pe[:], sss, MULT)
                            nc.vector.tensor_tensor(t4[:], po[:], cbs, MULT)
                            nc.vector.tensor_tensor(oo[:], t3[:], t4[:], ADD)
                            dstb = dst[nb]
                            tsl = ds(nto, STRIPE)
                            nc.sync.dma_start(
                                dstb[2 * a][0:64, tsl], oe[0:64, :])
                            nc.sync.dma_start(
                                dstb[2 * a + 1][0:64, tsl], oe[64:128, :])
                            nc.sync.dma_start(
                                dstb[2 * a][64:128, tsl], oo[0:64, :])
                            nc.sync.dma_start(
                                dstb[2 * a + 1][64:128, tsl], oo[64:128, :])
                    # v^T: x tiles stationary -> psum [tok-part, 512 feats]
                    for tt in range(STRIPE // P):
                        g = (STRIPE // P) * n + tt   # global token tile
                        bb, kto = divmod(g, S // P)
                        pv = vps.tile([P, FPC], f32, tag="pv", name="pv")
                        for kl in range(KO):
                            nc.tensor.matmul(
                                pv[:], xs[kl][:, ts(tt, P)], wvT_sb[:, kl],
                                start=(kl == 0), stop=(kl == KO - 1),
                            )
                        nc.scalar.activation(v_sb[:, bb, kto], pv[:], COPY)
                    if n == 2:
                        # batch-0 data is complete: preload unit 0's tiles
                        # now so attention starts the moment phase 1 ends
                        kh0 = prep.tile([P, S], bf16, tag="kh0", name="kh0")
                        nc.sync.dma_start(kh0[:], k_d[0][0])
                        qh0 = prep.tile([P, S], bf16, tag="qh0", name="qh0")
                        nc.sync.dma_start(qh0[:], q_d[0][0])

            # ---------- Phase 3: attention per (head, batch) + AllToAll -----
            with tc.tile_pool(name="bridge", bufs=1) as brp, \
                 tc.tile_pool(name="p4_part", bufs=32) as p4p:
              at2 = [
                  brp.tile([P, N_CORES, 512], bf16, tag=f"at2_{j}",
                           name=f"at2_{j}")
                  for j in range(HPC)
              ]
              parts = [
                  p4p.tile([P, 512], bf16, tag="opart", name="op")
                  for _ in range(DIM // P)
              ]
              with tc.tile_pool(name="p3_kqv", bufs=3) as hp, \
                 tc.tile_pool(name="p3_exp", bufs=10) as ep, \
                 tc.tile_pool(name="p3_acc", bufs=2) as accp, \
                 tc.tile_pool(name="p3_o", bufs=4) as aop, \
                 tc.tile_pool(name="p3_ps_s", bufs=2, space="PSUM") as sps, \
                 tc.tile_pool(name="p3_ps_o", bufs=1, space="PSUM") as ops, \
                 tc.tile_pool(name="p3_ps_d", bufs=1, space="PSUM") as dps, \
                 tc.tile_pool(name="p4_w", bufs=4) as wop, \
                 tc.tile_pool(name="p4_ps", bufs=2, space="PSUM") as p4ps:
                units = [(h, b) for h in range(HPC) for b in range(B)]
                loads = {}
                unit_ao = {}

                def emit_load(i):
                    h, b = units[i]
                    kh = hp.tile([P, S], bf16, tag="kh", name="kh")
                    nc.sync.dma_start(kh[:], k_d[b][h])
                    qh = hp.tile([P, S], bf16, tag="qh", name="qh")
                    nc.sync.dma_start(qh[:], q_d[b][h])
                    loads[i] = (kh, qh)

                loads[0] = (kh0, qh0)  # preloaded during phase 1
                emit_load(1)
                def emit_gather(j):
                    # tiny WAW write (overwritten by the gather): pins pass j
                    # after unit 2j+2's mid-unit progress, so the scheduler
                    # cannot order pass-j matmuls where the hardware would
                    # stall waiting on this AllToAll
                    if 2 * j + 2 in unit_ao:
                        nc.gpsimd.dma_start(
                            at2[j][0:1, 0, 0:64],
                            unit_ao[2 * j + 2][0:1, 0:64])
                    nc.gpsimd.dma_start(at2[j][:], cco3[j])

                for i, (h, b) in enumerate(units):
                    # prefetch two units ahead so the loads hit the DMA
                    # queues before this head's AllToAll monopolizes them
                    if i + 2 < len(units):
                        emit_load(i + 2)
                    if i >= 3 and i % 2 == 1:
                        emit_gather((i - 3) // 2)
                    kh, qh = loads.pop(i)
                    for qt in range(4):  # 512-token chunks within batch
                        ets = []
                        for k2 in range(S // P // 2):  # ktok tile pairs
                            ps_s = sps.tile([P, 1024], f32, tag="s",
                                            name="ps_s")
                            for kk in range(2):
                                kt = 2 * k2 + kk
                                nc.tensor.matmul(
                                    ps_s[:, ts(kk, 512)],
                                    kh[:, ts(kt, P)], qh[:, ts(qt, 512)],
                                    start=True, stop=True,
                                )
                            et = ep.tile([P, 1024], bf16, tag="e",
                                         name="et")
                            nc.scalar.activation(
                                et[:], ps_s[:], EXP, scale=SCALE
                            )
                            ets.append(et)
                        ps_o = ops.tile([P, 512], f32, tag="o", name="ps_o")
                        for kt in range(S // P):
                            nc.tensor.matmul(
                                ps_o[:], v_sb[:, b, kt, ds(h * P, P)],
                                ets[kt // 2][:, ts(kt % 2, 512)],
                                start=(kt == 0), stop=(kt == S // P - 1),
                            )
                        # softmax denominator: sum the 16 exp chunks on DVE
                        # (f32), then one 128-partition reduce+broadcast
                        # matmul against all-ones (vs 16 matmuls)
                        acc = accp.tile([P, 1024], f32, tag="acc", name="acc")
                        nc.vector.tensor_tensor(
                            acc[:], ets[0][:], ets[1][:], ADD)
                        for k2 in range(2, S // P // 2):
                            nc.vector.tensor_tensor(
                                acc[:], acc[:], ets[k2][:], ADD)
                        accf = accp.tile([P, 512], bf16, tag="accf",
                                         name="accf")
                        nc.vector.tensor_tensor(
                            accf[:], acc[:, 0:512], acc[:, 512:1024], ADD)
                        ps_d = dps.tile([P, 512], f32, tag="d", name="ps_d")
                        nc.tensor.matmul(
                            ps_d[:], ones_sb[:], accf[:],
                            start=True, stop=True,
                        )
                        rec = aop.tile([P, 512], f32, tag="rec", name="rec")
                        nc.vector.reciprocal_approx_fast(rec[:], ps_d[:])
                        ao = aop.tile([P, 512], bf16, tag="ao", name="ao")
                        nc.vector.tensor_tensor(ao[:], ps_o[:], rec[:], MULT)
                        nc.sync.dma_start(
                            cci3[h][:, 4 * b + qt, :], ao[:]
                        )
                        if qt == 1:
                            unit_ao[i] = ao
                    if b == 1:
                        # Fence: the collective freezes the DMA rings for its
                        # whole window, so any in-flight attention load would
                        # stall the pipeline. These tiny gpsimd-queue probe
                        # reads of the next-next unit's tiles delay the
                        # collective's issue until those loads have landed.
                        for ii in (i + 1, i + 2):
                            if ii not in loads:
                                continue
                            nkh, nqh = loads[ii]
                            nc.gpsimd.dma_start(probe_d[:], nkh[0:1, 0:64])
                            nc.gpsimd.dma_start(probe_d[:], nqh[0:1, 0:64])
                        # all 8 token-chunks of head h written -> redistribute
                        nc.gpsimd.collective_compute(
                            "AllToAll",
                            mybir.AluOpType.bypass,
                            replica_groups=[list(range(N_CORES))],
                            ins=[cc_in[h][:]],
                            outs=[cc_out[h][:]],
                        )
                        # GPSIMD queue: this gather waits on the collective,
                        # and on the SP/ACT sequencers that wait would
                        # head-of-line-block the DMA issues / exps behind it,
                        # freezing the attention pipeline for the whole A2A.
                        # The GPSIMD sequencer only runs the collectives, so
                        # the wait is harmless there.
                        if h == HPC - 1:
                            emit_gather(h)

                # ---- Phase 4 passes j=0..2: emitted last AND explicitly
                # deprioritized so the scheduler can never order them ahead
                # of attention work in the static PE stream (pass j blocks
                # on AllToAll j and would head-of-line-block the PE there)
                with tc.high_priority(offset=-1_000_000):
                    for j in range(HPC - 1):
                        for nt in range(DIM // P):
                            psum = p4ps.tile([P, 512], f32, tag="ops",
                                             name="psum")
                            w6 = wop.tile([P, N_CORES, P], bf16, tag="w6",
                                          name="w6")
                            nc.sync.dma_start(
                                w6[:], woH.ap()[nt][:, j, :, :])
                            for g in range(N_CORES):
                                nc.tensor.matmul(
                                    psum[:], w6[:, g], at2[j][:, g],
                                    start=(g == 0), stop=(g == N_CORES - 1),
                                )
                            if j == 0:
                                nc.scalar.activation(
                                    parts[nt][:], psum[:], COPY)
                            else:
                                nc.vector.tensor_tensor(
                                    parts[nt][:], psum[:], parts[nt][:], ADD
                                )

              # ---- Phase 4 final pass j=3: runs after attention pools close
              with tc.tile_pool(name="p4_w2", bufs=4) as wop2, \
                   tc.tile_pool(name="p4_s", bufs=4) as osp, \
                   tc.tile_pool(name="p4_ps2", bufs=4, space="PSUM") as opp2:
                for nt in range(DIM // P):
                    psum = opp2.tile([P, 512], f32, tag="ops2", name="psum2")
                    w2 = wop2.tile([P, N_CORES, P], bf16, tag="w2", name="w2")
                    nc.sync.dma_start(w2[:], woH.ap()[nt][:, HPC - 1, :, :])
                    for g in range(N_CORES):
                        nc.tensor.matmul(
                            psum[:], w2[:, g], at2[HPC - 1][:, g],
                            start=(g == 0), stop=(g == N_CORES - 1),
                        )
                    ob = osp.tile([P, 512], f32, tag="ob", name="ob")
                    nc.vector.tensor_tensor(ob[:], psum[:], parts[nt][:], ADD)
                    nc.sync.dma_start(oe3[:, nt], ob[:])

    nc.compile()
    return nc


def _prep_inputs(x, freqs_cos, freqs_sin, wq, wk, wv, wo):
    import ml_dtypes
    nbf = ml_dtypes.bfloat16

    x = np.asarray(x, dtype=np.float32)
    fc = np.asarray(freqs_cos, dtype=np.float32)
    fs = np.asarray(freqs_sin, dtype=np.float32)
    wq = np.asarray(wq, dtype=np.float32)
    wk = np.asarray(wk, dtype=np.float32)
    wv = np.asarray(wv, dtype=np.float32)
    wo = np.asarray(wo, dtype=np.float32)

    # rope tables [128, S]: row r uses pair index r%64 (rows 0:64 = head 2a,
    # 64:128 = head 2a+1 of each pair-tile)
    cb = np.ascontiguousarray(np.tile(fc.T, (2, 1)))   # [128, S]
    sst = np.ascontiguousarray(np.tile(fs.T, (2, 1)))  # [128, S]
    ones = np.ones((P, P), dtype=nbf)

    xTf = np.ascontiguousarray(x.reshape(TOK, DIM).T.astype(nbf))

    # feature order within a core's 512 q/k features: pair-tiles
    # [h2a evens | h2a+1 evens], [h2a odds | h2a+1 odds]
    order = []
    for a in range(HPC // 2):
        for par in range(2):
            for hh in (2 * a, 2 * a + 1):
                order.extend(hh * P + 2 * i + par for i in range(P // 2))
    order = np.array(order)

    def pack_qk(w, rows):
        # [512 out, 4096 in] -> reorder feats -> [m 4][128 in][32 kl][128 out]
        wr = w[rows][order]                      # [512 out(e/o), 4096 in]
        wT = wr.T                                # [4096 in, 512 out]
        return np.ascontiguousarray(
            wT.reshape(KO, P, HPC, P).transpose(2, 1, 0, 3).astype(nbf)
        )

    # wo.T [feat, dout] -> [nt 32, p 128, j 4, g 8, d 128]
    woHf = np.ascontiguousarray(
        wo.T.reshape(N_CORES, HPC, P, DIM // P, P).transpose(3, 2, 1, 0, 4)
        .astype(nbf)
    )
    in_maps = []
    for c in range(N_CORES):
        rows = slice(FPC * c, FPC * (c + 1))
        wvTf = np.ascontiguousarray(
            wv[rows].T.reshape(KO, P, FPC).transpose(1, 0, 2).astype(nbf)
        )
        in_maps.append({
            "xT": xTf,
            "wqH": pack_qk(wq, rows),
            "wkH": pack_qk(wk, rows),
            "wvT": wvTf,
            "woH": woHf,
            "cb": cb,
            "ss": sst,
            "ones": ones,
        })
    return in_maps


def _gather(results):
    y = np.empty((B, S, DIM), dtype=np.float32)
    for c in range(N_CORES):
        b, r = divmod(c, N_CORES // B)
        o = results[c]["out"]  # [4096 dout, 512 tok]
        y[b, 512 * r:512 * (r + 1), :] = o.T
    return y


def kernel(x, start_pos, freqs_cos, freqs_sin, wq, wk, wv, wo, trace=False):
    if "nc" not in _CACHE:
        _CACHE["nc"] = _build()
    nc = _CACHE["nc"]
    in_maps = _prep_inputs(x, freqs_cos, freqs_sin, wq, wk, wv, wo)
    res = run_bass_kernel_spmd(
        nc, in_maps, core_ids=list(range(N_CORES)), trace=trace
    )
    _CACHE["last_result"] = res
    return _gather(res.results)


# revision 35
# speedup vs baseline: 1.0054x; 1.0054x over previous
"""Trainium2 Bass kernel for nn_Attention (llama-style attention layer).

Full inputs in, full output out. 8-way tensor-parallel over heads (4 heads
per core, both batches on every core). All matmul operands in bf16 (fp32
PSUM accumulation), which halves HBM traffic and weight-load (LDWEIGHTS)
time vs f32r — the f32r baseline was LDWEIGHTS-bound at ~263ns per 512-row
matmul; bf16 runs at the ~213ns roofline.

  - merged q/k projections per head-pair with RoPE evaluated elementwise on
    DVE straight out of PSUM (features pre-reordered [evens|odds] host-side,
    so no permutation matmul / cross-partition shuffle is needed)
  - v projected directly in [token, feature] layout (x tiles as the matmul
    stationary) so no PE transposes are needed for the attention AV matmul
  - per-head attention in [feat, tok] layout, softmax denominator via
    all-ones matmul, normalization on eviction
  - per-head AllToAll (8 cores, bf16) redistributes attention output from
    head-sharding to token-sharding, overlapped with later heads
  - output projection streams the full wo in four per-head-index passes,
    each gated only on its own AllToAll, accumulating into SBUF partials;
    the passes are emitted last so they fill tensor-engine gaps
"""
import os
import sys

sys.path.insert(0, "/opt/trn_rl_repo")

import numpy as np

import concourse.bass as bass
import concourse.mybir as mybir
import concourse.tile as tile
from concourse import bacc
from concourse.bass import ds, ts
from concourse.bass_utils import run_bass_kernel_spmd

DIM = 4096
N_HEADS = 32
HEAD_DIM = 128
B, S = 2, 2048
TOK = B * S                   # 4096 global tokens
N_CORES = 8
HPC = N_HEADS // N_CORES      # heads per core = 4
FPC = HPC * HEAD_DIM          # features per core = 512
P = 128
KO = DIM // P                 # 32 k-tiles over the model dim
STRIPE = 1024
NSTRIPE = TOK // STRIPE       # 4 projection stripes of 1024 tokens
SCALE = 1.0 / float(np.sqrt(HEAD_DIM))

f32 = mybir.dt.float32
bf16 = mybir.dt.bfloat16
EXP = mybir.ActivationFunctionType.Exp
COPY = mybir.ActivationFunctionType.Copy
MULT = mybir.AluOpType.mult
ADD = mybir.AluOpType.add
SUB = mybir.AluOpType.subtract

_CACHE = {}


def _build():
    nc = bacc.Bacc(
        "TRN2", target_bir_lowering=False, debug=False, num_devices=N_CORES
    )

    xT = nc.dram_tensor("xT", [DIM, TOK], bf16, kind="ExternalInput")
    # q/k weights: per-head-pair stationary tiles, features in [e|o] order
    wqH = nc.dram_tensor("wqH", [HPC, P, KO, P], bf16, kind="ExternalInput")
    wkH = nc.dram_tensor("wkH", [HPC, P, KO, P], bf16, kind="ExternalInput")
    # v weights transposed: [in-part, ktile, out-feats] (moving operand)
    wvT = nc.dram_tensor("wvT", [P, KO, FPC], bf16, kind="ExternalInput")
    woH = nc.dram_tensor("woH", [DIM // P, P, HPC, N_CORES, P], bf16,
                         kind="ExternalInput")
    cb_d = nc.dram_tensor("cb", [P, S], f32, kind="ExternalInput")
    ss_d = nc.dram_tensor("ss", [P, S], f32, kind="ExternalInput")
    ones_d = nc.dram_tensor("ones", [P, P], bf16, kind="ExternalInput")
    out_e = nc.dram_tensor("out", [DIM, TOK // N_CORES], f32, kind="ExternalOutput")

    xT3 = xT.ap().rearrange("(ko p) t -> p ko t", p=P)       # [128, 32, 4096]
    oe3 = out_e.ap().rearrange("(no p) t -> p no t", p=P)    # [128, 32, 512]

    with tile.TileContext(nc) as tc:
        with tc.tile_pool(name="dram", bufs=1, space="DRAM") as drp, \
             tc.tile_pool(name="const", bufs=1) as constp, \
             tc.tile_pool(name="preload", bufs=1) as prep:
            # q/k post-rope, [head][feat e|o][tok-within-batch], bf16;
            # split per batch so batch-0 attention loads don't dep-wait on
            # the batch-1 projection stripes
            q_d = [drp.tile([HPC, P, S], bf16, tag=f"q_d{b}", name=f"q_d{b}")
                   for b in range(B)]
            k_d = [drp.tile([HPC, P, S], bf16, tag=f"k_d{b}", name=f"k_d{b}")
                   for b in range(B)]
            # v stays resident in SBUF for the whole kernel: no DRAM
            # round-trip, and nothing v-related can be stranded in the DMA
            # rings when a collective freezes them
            v_sb = prep.tile([P, B, S // P, FPC], bf16, tag="v_sb",
                             name="v_sb")
            cc_in = [
                drp.tile([N_CORES * P, 512], bf16, tag=f"cci{j}", name=f"cci{j}")
                for j in range(HPC)
            ]
            cc_out = [
                drp.tile([N_CORES * P, 512], bf16, tag=f"cco{j}", name=f"cco{j}")
                for j in range(HPC)
            ]
            cci3 = [c[:].rearrange("(r p) t -> p r t", p=P) for c in cc_in]
            cco3 = [c[:].rearrange("(g p) t -> p g t", p=P) for c in cc_out]
            # scratch target for the tiny gpsimd probe-DMAs that fence each
            # AllToAll behind the next attention unit's loads
            probe_d = drp.tile([1, 64], bf16, tag="probe", name="probe_d")

            ones_sb = constp.tile([P, P], bf16, tag="ones", name="ones_sb")
            nc.sync.dma_start(ones_sb[:], ones_d.ap())

            # ---------- Phase 1: q/k projections (+RoPE) and v^T projection
            with tc.tile_pool(name="p1_rope", bufs=1) as ropep, \
                 tc.tile_pool(name="p1_wv", bufs=1) as wvp, \
                 tc.tile_pool(name="p1_x", bufs=34) as xp, \
                 tc.tile_pool(name="p1_w", bufs=3) as wtp, \
                 tc.tile_pool(name="p1_tmp", bufs=4) as tmpp, \
                 tc.tile_pool(name="p1_qko", bufs=4) as qkop, \
                 tc.tile_pool(name="p1_ps", bufs=3, space="PSUM") as qkps, \
                 tc.tile_pool(name="p1_psv", bufs=2, space="PSUM") as vps:
                cb_sb = ropep.tile([P, S], f32, tag="cb", name="cb_sb")
                ss_sb = ropep.tile([P, S], f32, tag="ss", name="ss_sb")
                wvT_sb = wvp.tile([P, KO, FPC], bf16, tag="wvT", name="wvT_sb")

                for n in range(NSTRIPE):  # 4 stripes of 1024 tokens
                    rtok = (STRIPE * n) % S   # rope tables repeat per batch
                    nb = n // (S // STRIPE)   # batch of this stripe
                    nto = (STRIPE * n) % S    # token offset within batch
                    pairlist = [(wH, dst, a)
                                for wH, dst in ((wqH, q_d), (wkH, k_d))
                                for a in range(HPC // 2)]
                    # interleave weight DMAs with the x-tile stream so each
                    # pair's weights land before its matmuls are reached
                    # (instead of FIFO-queuing behind 8MB of x)
                    wts = {}

                    def emit_wt(pi):
                        wH, _, a = pairlist[pi]
                        wt_e = wtp.tile([P, KO, P], bf16, tag="wt",
                                        name="wt_e")
                        wt_o = wtp.tile([P, KO, P], bf16, tag="wt",
                                        name="wt_o")
                        nc.sync.dma_start(wt_e[:], wH.ap()[2 * a])
                        nc.sync.dma_start(wt_o[:], wH.ap()[2 * a + 1])
                        wts[pi] = (wt_e, wt_o)

                    emit_wt(0)
                    xs = [
                        xp.tile([P, STRIPE], bf16, tag="xsl", name="xs")
                        for _ in range(KO)
                    ]
                    for kl in range(KO):
                        nc.sync.dma_start(xs[kl][:], xT3[:, kl, ts(n, STRIPE)])
                        if n == 0 and kl in (7, 15, 23):
                            emit_wt(kl // 8 + 1)
                    if n == 0:
                        # emitted after the first stripe's x/weight DMAs so
                        # they don't delay the first matmul
                        nc.sync.dma_start(cb_sb[:], cb_d.ap())
                        nc.sync.dma_start(ss_sb[:], ss_d.ap())
                        nc.sync.dma_start(wvT_sb[:], wvT.ap())
                    for pi, (wH, dst, a) in enumerate(pairlist):
                            pe = qkps.tile([P, STRIPE], f32, tag="qk",
                                           name="pe")
                            po = qkps.tile([P, STRIPE], f32, tag="qk",
                                           name="po")
                            if pi not in wts:
                                emit_wt(pi)
                            wt_e, wt_o = wts.pop(pi)
                            for kl in range(KO):
                                for c in range(2):
                                    nc.tensor.matmul(
                                        pe[:, ts(c, 512)], wt_e[:, kl],
                                        xs[kl][:, ts(c, 512)],
                                        start=(kl == 0), stop=(kl == KO - 1),
                                    )
                                for c in range(2):
                                    nc.tensor.matmul(
                                        po[:, ts(c, 512)], wt_o[:, kl],
                                        xs[kl][:, ts(c, 512)],
                                        start=(kl == 0), stop=(kl == KO - 1),
                                    )
                            # RoPE: rows 0:64 head 2a, 64:128 head 2a+1
                            # (pe = even-index feats, po = odd-index feats)
                            cbs = cb_sb[:, ds(rtok, STRIPE)]
                            sss = ss_sb[:, ds(rtok, STRIPE)]
                            t1 = tmpp.tile([P, STRIPE], f32, tag="t", name="t1")
                            t2 = tmpp.tile([P, STRIPE], f32, tag="t", name="t2")
                            t3 = tmpp.tile([P, STRIPE], f32, tag="t", name="t3")
                            t4 = tmpp.tile([P, STRIPE], f32, tag="t", name="t4")
                            oe = qkop.tile([P, STRIPE], bf16, tag="o", name="oe")
                            oo = qkop.tile([P, STRIPE], bf16, tag="o", name="oo")
                            nc.vector.tensor_tensor(t1[:], pe[:], cbs, MULT)
                            nc.vector.tensor_tensor(t2[:], po[:], sss, MULT)
                            nc.vector.tensor_tensor(oe[:], t1[:], t2[:], SUB)
                            nc.vector.tensor_tensor(t3[:], pe[:], sss, MULT)
                            nc.vector.tensor_tensor(t4[:], po[:], cbs, MULT)
                            nc.vector.tensor_tensor(oo[:], t3[:], t4[:], ADD)
                            dstb = dst[nb]
                            tsl = ds(nto, STRIPE)
                            nc.sync.dma_start(
                                dstb[2 * a][0:64, tsl], oe[0:64, :])
                            nc.sync.dma_start(
                                dstb[2 * a + 1][0:64, tsl], oe[64:128, :])
                            nc.sync.dma_start(
                                dstb[2 * a][64:128, tsl], oo[0:64, :])
                            nc.sync.dma_start(
                                dstb[2 * a + 1][64:128, tsl], oo[64:128, :])
                    # v^T: x tiles stationary -> psum [tok-part, 512 feats]
                    for tt in range(STRIPE // P):
                        g = (STRIPE // P) * n + tt   # global token tile
                        bb, kto = divmod(g, S // P)
                        pv = vps.tile([P, FPC], f32, tag="pv", name="pv")
                        for kl in range(KO):
                            nc.tensor.matmul(
                                pv[:], xs[kl][:, ts(tt, P)], wvT_sb[:, kl],
                                start=(kl == 0), stop=(kl == KO - 1),
                            )
                        nc.scalar.activation(v_sb[:, bb, kto], pv[:], COPY)
                    if n == 2:
                        # batch-0 data is complete: preload unit 0's tiles
                        # now so attention starts the moment phase 1 ends
                        kh0 = prep.tile([P, S], bf16, tag="kh0", name="kh0")
                        nc.sync.dma_start(kh0[:], k_d[0][0])
                        qh0 = prep.tile([P, S], bf16, tag="qh0", name="qh0")
                        nc.sync.dma_start(qh0[:], q_d[0][0])

            # ---------- Phase 3: attention per (head, batch) + AllToAll -----
            with tc.tile_pool(name="bridge", bufs=1) as brp, \
                 tc.tile_pool(name="p4_part", bufs=32) as p4p:
              at2g = [
                  [brp.tile([P, 512], bf16, tag=f"at2_{j}_{g}",
                            name=f"at2_{j}_{g}") for g in range(N_CORES)]
                  for j in range(HPC)
              ]
              parts = [
                  p4p.tile([P, 512], bf16, tag="opart", name="op")
                  for _ in range(DIM // P)
              ]
              with tc.tile_pool(name="p3_kqv", bufs=3) as hp, \
                 tc.tile_pool(name="p3_exp", bufs=10) as ep, \
                 tc.tile_pool(name="p3_acc", bufs=2) as accp, \
                 tc.tile_pool(name="p3_o", bufs=4) as aop, \
                 tc.tile_pool(name="p3_ps_s", bufs=2, space="PSUM") as sps, \
                 tc.tile_pool(name="p3_ps_o", bufs=1, space="PSUM") as ops, \
                 tc.tile_pool(name="p3_ps_d", bufs=1, space="PSUM") as dps, \
                 tc.tile_pool(name="p4_w", bufs=4) as wop, \
                 tc.tile_pool(name="p4_ps", bufs=2, space="PSUM") as p4ps:
                units = [(h, b) for h in range(HPC) for b in range(B)]
                loads = {}
                unit_ao = {}

                def emit_load(i):
                    h, b = units[i]
                    kh = hp.tile([P, S], bf16, tag="kh", name="kh")
                    nc.sync.dma_start(kh[:], k_d[b][h])
                    qh = hp.tile([P, S], bf16, tag="qh", name="qh")
                    nc.sync.dma_start(qh[:], q_d[b][h])
                    loads[i] = (kh, qh)

                loads[0] = (kh0, qh0)  # preloaded during phase 1
                emit_load(1)
                for i, (h, b) in enumerate(units):
                    # prefetch two units ahead so the loads hit the DMA
                    # queues before this head's AllToAll monopolizes them
                    if i + 2 < len(units):
                        emit_load(i + 2)
                    kh, qh = loads.pop(i)
                    for qt in range(4):  # 512-token chunks within batch
                        ets = []
                        for k2 in range(S // P // 2):  # ktok tile pairs
                            ps_s = sps.tile([P, 1024], f32, tag="s",
                                            name="ps_s")
                            for kk in range(2):
                                kt = 2 * k2 + kk
                                nc.tensor.matmul(
                                    ps_s[:, ts(kk, 512)],
                                    kh[:, ts(kt, P)], qh[:, ts(qt, 512)],
                                    start=True, stop=True,
                                )
                            et = ep.tile([P, 1024], bf16, tag="e",
                                         name="et")
                            nc.scalar.activation(
                                et[:], ps_s[:], EXP, scale=SCALE
                            )
                            ets.append(et)
                        ps_o = ops.tile([P, 512], f32, tag="o", name="ps_o")
                        for kt in range(S // P):
                            nc.tensor.matmul(
                                ps_o[:], v_sb[:, b, kt, ds(h * P, P)],
                                ets[kt // 2][:, ts(kt % 2, 512)],
                                start=(kt == 0), stop=(kt == S // P - 1),
                            )
                        # softmax denominator: sum the 16 exp chunks on DVE
                        # (f32), then one 128-partition reduce+broadcast
                        # matmul against all-ones (vs 16 matmuls)
                        acc = accp.tile([P, 1024], f32, tag="acc", name="acc")
                        nc.vector.tensor_tensor(
                            acc[:], ets[0][:], ets[1][:], ADD)
                        for k2 in range(2, S // P // 2):
                            nc.vector.tensor_tensor(
                                acc[:], acc[:], ets[k2][:], ADD)
                        accf = accp.tile([P, 512], bf16, tag="accf",
                                         name="accf")
                        nc.vector.tensor_tensor(
                            accf[:], acc[:, 0:512], acc[:, 512:1024], ADD)
                        ps_d = dps.tile([P, 512], f32, tag="d", name="ps_d")
                        nc.tensor.matmul(
                            ps_d[:], ones_sb[:], accf[:],
                            start=True, stop=True,
                        )
                        rec = aop.tile([P, 512], f32, tag="rec", name="rec")
                        nc.vector.reciprocal_approx_fast(rec[:], ps_d[:])
                        ao = aop.tile([P, 512], bf16, tag="ao", name="ao")
                        nc.vector.tensor_tensor(ao[:], ps_o[:], rec[:], MULT)
                        nc.sync.dma_start(
                            cci3[h][:, 4 * b + qt, :], ao[:]
                        )
                        if qt == 1:
                            unit_ao[i] = ao
                    if b == 1:
                        # Fence: the collective freezes the DMA rings for its
                        # whole window, so any in-flight attention load would
                        # stall the pipeline. These tiny gpsimd-queue probe
                        # reads of the next-next unit's tiles delay the
                        # collective's issue until those loads have landed.
                        for ii in (i + 1, i + 2):
                            if ii not in loads:
                                continue
                            nkh, nqh = loads[ii]
                            nc.gpsimd.dma_start(probe_d[:], nkh[0:1, 0:64])
                            nc.gpsimd.dma_start(probe_d[:], nqh[0:1, 0:64])
                        # all 8 token-chunks of head h written -> redistribute
                        nc.gpsimd.collective_compute(
                            "AllToAll",
                            mybir.AluOpType.bypass,
                            replica_groups=[list(range(N_CORES))],
                            ins=[cc_in[h][:]],
                            outs=[cc_out[h][:]],
                        )
                        # GPSIMD queue: this gather waits on the collective,
                        # and on the SP/ACT sequencers that wait would
                        # head-of-line-block the DMA issues / exps behind it,
                        # freezing the attention pipeline for the whole A2A.
                        # The GPSIMD sequencer only runs the collectives, so
                        # the wait is harmless there.
                        # per-g gather tiles: pass h's first matmuls only
                        # wait for the first 128KB instead of the whole 1MB
                        # (the gpsimd SW-DGE gather is slow, ~7us for 1MB)
                        for g in range(N_CORES):
                            nc.gpsimd.dma_start(
                                at2g[h][g][:], cco3[h][:, g, :])

                # ---- Phase 4 passes j=0..2: emitted last AND explicitly
                # deprioritized so the scheduler can never order them ahead
                # of attention work in the static PE stream (pass j blocks
                # on AllToAll j and would head-of-line-block the PE there)
                with tc.high_priority(offset=-1_000_000):
                    for j in range(HPC - 1):
                        for nt in range(DIM // P):
                            psum = p4ps.tile([P, 512], f32, tag="ops",
                                             name="psum")
                            w6 = wop.tile([P, N_CORES, P], bf16, tag="w6",
                                          name="w6")
                            nc.sync.dma_start(
                                w6[:], woH.ap()[nt][:, j, :, :])
                            for g in range(N_CORES):
                                nc.tensor.matmul(
                                    psum[:], w6[:, g], at2g[j][g][:],
                                    start=(g == 0), stop=(g == N_CORES - 1),
                                )
                            if j == 0:
                                nc.scalar.activation(
                                    parts[nt][:], psum[:], COPY)
                            else:
                                nc.vector.tensor_tensor(
                                    parts[nt][:], psum[:], parts[nt][:], ADD
                                )

              # ---- Phase 4 final pass j=3: runs after attention pools close
              with tc.tile_pool(name="p4_w2", bufs=4) as wop2, \
                   tc.tile_pool(name="p4_s", bufs=4) as osp, \
                   tc.tile_pool(name="p4_ps2", bufs=4, space="PSUM") as opp2:
                for nt in range(DIM // P):
                    psum = opp2.tile([P, 512], f32, tag="ops2", name="psum2")
                    w2 = wop2.tile([P, N_CORES, P], bf16, tag="w2", name="w2")
                    nc.sync.dma_start(w2[:], woH.ap()[nt][:, HPC - 1, :, :])
                    for g in range(N_CORES):
                        nc.tensor.matmul(
                            psum[:], w2[:, g], at2g[HPC - 1][g][:],
                            start=(g == 0), stop=(g == N_CORES - 1),
                        )
                    ob = osp.tile([P, 512], f32, tag="ob", name="ob")
                    nc.vector.tensor_tensor(ob[:], psum[:], parts[nt][:], ADD)
                    nc.sync.dma_start(oe3[:, nt], ob[:])

    nc.compile()
    return nc


def _prep_inputs(x, freqs_cos, freqs_sin, wq, wk, wv, wo):
    import ml_dtypes
    nbf = ml_dtypes.bfloat16

    x = np.asarray(x, dtype=np.float32)
    fc = np.asarray(freqs_cos, dtype=np.float32)
    fs = np.asarray(freqs_sin, dtype=np.float32)
    wq = np.asarray(wq, dtype=np.float32)
    wk = np.asarray(wk, dtype=np.float32)
    wv = np.asarray(wv, dtype=np.float32)
    wo = np.asarray(wo, dtype=np.float32)

    # rope tables [128, S]: row r uses pair index r%64 (rows 0:64 = head 2a,
    # 64:128 = head 2a+1 of each pair-tile)
    cb = np.ascontiguousarray(np.tile(fc.T, (2, 1)))   # [128, S]
    sst = np.ascontiguousarray(np.tile(fs.T, (2, 1)))  # [128, S]
    ones = np.ones((P, P), dtype=nbf)

    xTf = np.ascontiguousarray(x.reshape(TOK, DIM).T.astype(nbf))

    # feature order within a core's 512 q/k features: pair-tiles
    # [h2a evens | h2a+1 evens], [h2a odds | h2a+1 odds]
    order = []
    for a in range(HPC // 2):
        for par in range(2):
            for hh in (2 * a, 2 * a + 1):
                order.extend(hh * P + 2 * i + par for i in range(P // 2))
    order = np.array(order)

    def pack_qk(w, rows):
        # [512 out, 4096 in] -> reorder feats -> [m 4][128 in][32 kl][128 out]
        wr = w[rows][order]                      # [512 out(e/o), 4096 in]
        wT = wr.T                                # [4096 in, 512 out]
        return np.ascontiguousarray(
            wT.reshape(KO, P, HPC, P).transpose(2, 1, 0, 3).astype(nbf)
        )

    # wo.T [feat, dout] -> [nt 32, p 128, j 4, g 8, d 128]
    woHf = np.ascontiguousarray(
        wo.T.reshape(N_CORES, HPC, P, DIM // P, P).transpose(3, 2, 1, 0, 4)
        .astype(nbf)
    )
    in_maps = []
    for c in range(N_CORES):
        rows = slice(FPC * c, FPC * (c + 1))
        wvTf = np.ascontiguousarray(
            wv[rows].T.reshape(KO, P, FPC).transpose(1, 0, 2).astype(nbf)
        )
        in_maps.append({
            "xT": xTf,
            "wqH": pack_qk(wq, rows),
            "wkH": pack_qk(wk, rows),
            "wvT": wvTf,
            "woH": woHf,
            "cb": cb,
            "ss": sst,
            "ones": ones,
        })
    return in_maps


def _gather(results):
    y = np.empty((B, S, DIM), dtype=np.float32)
    for c in range(N_CORES):
        b, r = divmod(c, N_CORES // B)
        o = results[c]["out"]  # [4096 dout, 512 tok]
        y[b, 512 * r:512 * (r + 1), :] = o.T
    return y


def kernel(x, start_pos, freqs_cos, freqs_sin, wq, wk, wv, wo, trace=False):
    if "nc" not in _CACHE:
        _CACHE["nc"] = _build()
    nc = _CACHE["nc"]
    in_maps = _prep_inputs(x, freqs_cos, freqs_sin, wq, wk, wv, wo)
    res = run_bass_kernel_spmd(
        nc, in_maps, core_ids=list(range(N_CORES)), trace=trace
    )
    _CACHE["last_result"] = res
    return _gather(res.results)
